# revision 35
# baseline (speedup 1.0000x reference)
# Trainium2 Bass kernel for the KerasLMU problem.
#
# Math: per time step t (T=1024 steps),
#   u_t = x_t @ e_x                       (B,1)
#   m_t = m_{t-1} @ A.T + b_row * u_t     (B,256)   -- linear recurrence
#   h_t = lrelu(x_t @ W_x + h_{t-1} @ W_h.T + m_t @ W_m.T)
#
# Reformulation: m_t = sum_k A^k b u_{t-k} (causal convolution), so
#   c_t := x_t @ W_x + m_t @ W_m.T = x_t @ W_x + sum_k G[k] u_{t-k}
# with G[k] = W_m @ (A^k b) precomputed host-side in float64. The only
# sequential device work left is h_t = lrelu(c_t + h_{t-1} @ W_h.T).
#
# Device pipeline (per core, data-parallel over batch; 8 rows/core):
#   A: PE-transpose x -> xT (bf16)          [feat, (b,tau)]
#   B: u = e_x^T @ xT -> u_pad DRAM (bf16, 512 zeros prefix)
#   C: overlapping-window DMA -> ushr (128 shifted copies of u)
#   D: c^T = conv(G,u) + W_x^T xT, bf16 matmuls, f32 PSUM, c kept
#      RESIDENT IN SBUF as bf16 [128, 4, BC, T] (no DRAM round-trip)
#   E: 1024-step recurrence. Per step 16 [128x128]x[128x8] bf16
#      matmuls in a staggered 4-bank slot schedule so each PSUM bank
#      completes ~8 slots before its h-chunk is consumed next step,
#      hiding the PE->act->PE latency (~570ns). Leaky-ReLU alternates
#      between Scalar (Prelu) and Vector (scalar_tensor_tensor) so
#      neither engine serializes the chain. h blocks DMA to DRAM in
#      SBUF-native layout (contiguous 4KB descriptors); the final
#      [b, t, hid] permutation happens host-side in numpy.

import os
import sys

sys.path.insert(0, "/opt/trn_rl_repo")

import numpy as np
import ml_dtypes

import concourse.bass as bass
import concourse.tile as tile
from concourse import bacc, mybir
from concourse.bass_utils import run_bass_kernel_spmd

F32 = mybir.dt.float32
BF16 = mybir.dt.bfloat16
BF = ml_dtypes.bfloat16

NCORES = 8
BATCH = 64
BC = BATCH // NCORES          # batch rows per core = 8
FEAT = 128
HID = 512
ORDER = 256
TFULL = 1024
TBLK = 64                     # seq-loop steps per out-DMA block

# phase-E: the per-core batch of 8 is split into two independent 4-col
# streams, interleaved on the PE. While stream A's activation round-trip
# (~400ns of sem+act latency) is in flight, the PE runs stream B's 16
# matmuls, and vice versa -- the PE never idles, so the recurrence runs at
# PE issue rate (~32 matmuls/step) instead of act-latency rate.
SLOT2MK = [(mc, kc) for kc in range(4) for mc in range(4)]
BCH = BC // 2                 # cols per stream = 4

# module-level stash for test harness introspection
last_run_info = {}


def _dap(handle, offset, dims):
    """Build an explicit AP on a DRAM tensor: dims = [[step, count], ...]
    (element units; first dim pairs with the SBUF partition dim)."""
    base = handle[:]
    return bass.AP(tensor=base.tensor, offset=offset, ap=[list(d) for d in dims])


def build_nc(T=TFULL, tblk=TBLK):
    """Emit the per-core Bass/Tile program (SPMD; all cores identical)."""
    nblk = T // tblk
    BT = BC * T                       # rows of x per core
    nxt = BT // 128                   # 128-row x tiles
    th_n = T // 512 if T >= 512 else 1  # 512-wide tau halves in conv
    tw = min(T, 512)                  # conv tau tile width
    OWB = tblk * 4 * BC               # out elements per partition per block

    nc = bacc.Bacc(None, target_bir_lowering=False)
    x_d = nc.declare_dram_parameter("x", [BT, FEAT], F32, isOutput=False)
    whT_d = nc.declare_dram_parameter("whT", [HID, HID], BF16, isOutput=False)
    g_d = nc.declare_dram_parameter("g", [T, HID], BF16, isOutput=False)
    wx_d = nc.declare_dram_parameter("wx", [FEAT, HID], BF16, isOutput=False)
    ex_d = nc.declare_dram_parameter("ex", [FEAT, 1], BF16, isOutput=False)
    id_d = nc.declare_dram_parameter("ident", [128, 128], F32, isOutput=False)
    # out in SBUF-native block layout: row = blk*128 + p, col = dt*32+mc*8+b8
    out_d = nc.declare_dram_parameter("out", [nblk * 128, OWB], BF16,
                                      isOutput=True)

    UPADW = 512 + T                   # zeros(512) ++ u(T)
    upad_d = nc.dram_tensor("u_pad", [BC, UPADW], BF16)

    USHW = T + 384                    # Qi domain width
    KCN = T // 128                    # lag chunks

    with tile.TileContext(nc) as tc:
        with (
            tc.tile_pool(name="consts", bufs=1) as consts,
            tc.tile_pool(name="work", bufs=4) as work,
            tc.tile_pool(name="hout", bufs=2) as hpool,
            tc.tile_pool(name="psA", bufs=4, space="PSUM") as psA,
            tc.tile_pool(name="psS", bufs=4, space="PSUM") as psS,
        ):
            # ---- resident constants -------------------------------------
            # identity first: the x transposes (critical path) need it
            id_sb = consts.tile([128, 128], F32)
            nc.sync.dma_start(out=id_sb, in_=id_d[:, :])
            ex_sb = consts.tile([128, 1], BF16)
            nc.sync.dma_start(out=ex_sb, in_=ex_d[:, :])

            xT_sb = consts.tile([128, BT], BF16)    # x.T : [feat, (b,tau)]
            # one ushr tile per batch row: the conv for row b8 then depends
            # only on b8's window-DMA, not on all eight
            ushr = [consts.tile([128, USHW], BF16, name=f"ushr{b8}")
                    for b8 in range(BC)]
            c_sb = consts.tile([128, 4, BC, T], BF16)  # c^T resident
            zrow = consts.tile([1, 512], BF16)
            nc.vector.memset(zrow, 0.0)
            h0 = consts.tile([128, 4, BC], BF16)
            nc.vector.memset(h0, 0.0)

            # ---- phases A+B+C, pipelined per batch row ------------------
            # Per b8: stage b8's slice of x (one DMA), PE-transpose it,
            # u = x @ e_x, round-trip through DRAM into the reversed shift
            # matrix ushr[b8][p, Qi] = u_pad[b8][1 + Qi + p]. Row b8's DMAs
            # overlap row b8+1's transposes.
            rpb = T // 128                  # x tiles per batch row
            x_st = [consts.tile([128, rpb, 128], F32, name=f"xst{b8}")
                    for b8 in range(BC)]
            for b8 in range(BC):
                nc.sync.dma_start(
                    out=x_st[b8],
                    in_=_dap(x_d, b8 * T * FEAT,
                             [[FEAT, 128], [128 * FEAT, rpb], [1, FEAT]]))
            # weight loads issue after the x staging (x gates the critical
            # path; whT/g aren't needed until the conv/recurrence)
            whT_sb = consts.tile([128, 4, HID], BF16)
            for kc in range(4):
                nc.sync.dma_start(out=whT_sb[:, kc, :],
                                  in_=whT_d[kc * 128:(kc + 1) * 128, :])
            g_sb = consts.tile([128, KCN, HID], BF16)
            for kc in range(KCN):
                nc.sync.dma_start(out=g_sb[:, kc, :],
                                  in_=g_d[kc * 128:(kc + 1) * 128, :])
            wx_sb = consts.tile([128, HID], BF16)
            nc.sync.dma_start(out=wx_sb, in_=wx_d[:, :])
            for b8 in range(BC):
                for r in range(rpb):
                    ps = psA.tile([128, 128], F32, tag="ps")
                    nc.tensor.transpose(ps, x_st[b8][:, r, :], id_sb)
                    dst = xT_sb[:, (b8 * rpb + r) * 128:
                                (b8 * rpb + r + 1) * 128]
                    if r % 2 == 0:
                        nc.scalar.copy(dst, ps)
                    else:
                        nc.vector.tensor_copy(dst, ps)
                urow = work.tile([1, UPADW], BF16, tag="urow")
                nc.vector.tensor_copy(urow[:, 0:512], zrow)
                for th in range((T + 511) // 512):
                    w = min(512, T - th * 512)
                    ps = psA.tile([1, 512], F32, tag="ps")
                    nc.tensor.matmul(ps[:, :w], lhsT=ex_sb,
                                     rhs=xT_sb[:, b8 * T + th * 512:
                                               b8 * T + th * 512 + w],
                                     start=True, stop=True)
                    nc.scalar.copy(urow[:, 512 + th * 512:512 + th * 512 + w],
                                   ps[:, :w])
                nc.gpsimd.dma_start(out=upad_d[b8:b8 + 1, :], in_=urow)
                nc.sync.dma_start(
                    out=ushr[b8],
                    in_=_dap(upad_d, b8 * UPADW + 1, [[1, 128], [1, USHW]]))

            # ---- phase D: c^T = conv(G, u) + W_x^T @ x^T -> c_sb (SBUF) -
            # 512-column tau tiles. Lag chunk kc contributes only to taus
            # >= 128*kc (earlier taus fall in the zero prefix of u_pad), so
            # each matmul's tau range is trimmed. th=0 (taus 0..511, needed
            # before E step 0) runs in the prologue; th=1 groups drip into
            # phase E's PE idle gap at 1 instruction per step.
            pace = {"ap": None}

            def dgroup_gen(b8, jt, th, scalar_copy=False):
                ps = psA.tile([128, tw], F32, tag="ps",
                              name=f"cps{b8}_{jt}_{th}")
                if pace["ap"] is not None:
                    # artificial dep on the current E step's h tile: stops
                    # the scheduler from hoisting this group's matmuls into
                    # the first few steps (which overruns their idle gaps)
                    nc.vector.tensor_copy(ps[:, 0:1], pace["ap"])
                first = True
                kmax = min(KCN, 4 * th + tw // 128)
                for kc in range(kmax):
                    s0 = max(0, 128 * kc - 512 * th)
                    qi0 = 384 + 512 * th - 128 * kc
                    nc.tensor.matmul(
                        ps[:, s0:tw],
                        lhsT=g_sb[:, kc, jt * 128:(jt + 1) * 128],
                        rhs=ushr[b8][:, qi0 + s0:qi0 + tw],
                        start=first, stop=False)
                    first = False
                    yield
                nc.tensor.matmul(
                    ps, lhsT=wx_sb[:, jt * 128:(jt + 1) * 128],
                    rhs=xT_sb[:, b8 * T + th * 512:b8 * T + th * 512 + tw],
                    start=False, stop=True)
                yield
                if scalar_copy:
                    nc.scalar.copy(c_sb[:, jt, b8, th * 512:th * 512 + tw],
                                   ps)
                    yield
                else:
                    # split halves across scalar+vector so neither engine
                    # saturates when this lands inside a phase-E step
                    hw2 = tw // 2
                    o0 = th * 512
                    nc.scalar.copy(c_sb[:, jt, b8, o0:o0 + hw2],
                                   ps[:, 0:hw2])
                    yield
                    nc.vector.tensor_copy(c_sb[:, jt, b8, o0 + hw2:o0 + tw],
                                          ps[:, hw2:tw])
                    yield

            ev = 0
            for b8 in range(BC):
                for jt in range(4):
                    for _ in dgroup_gen(b8, jt, 0, scalar_copy=ev % 2 == 0):
                        pass
                    ev += 1
            dfill = []
            if th_n == 2:
                for b8 in range(BC):
                    for jt in range(4):
                        dfill.append(dgroup_gen(b8, jt, 1))

            def dfill_step():
                while dfill:
                    try:
                        next(dfill[0])
                        return
                    except StopIteration:
                        dfill.pop(0)

            # ---- phase E: sequential h recurrence -----------------------
            # Warm all psS banks once: a start=True pass clears the
            # pending-zero bits over our regions so the per-step matmuls
            # can run start=False and accumulate onto a prewritten c_t
            # (keeps the c add off the PE critical path).
            # Every per-stream object (PSUM step tile, h output tile) is a
            # SEPARATE tile: dependency tracking at any granularity then
            # cannot couple the two streams, so their act round-trips
            # overlap with the other stream's matmuls. PSUM step tiles are
            # padded to a full 2KB bank (a bank holds one active
            # accumulation group); bufs=2 per stream -> 4 banks.
            warm = [psS.tile([128, 4, 128], F32, tag=f"pss{s}", bufs=2,
                             name=f"warm{s}{i}")
                    for s in "AB" for i in range(2)]
            for mc in range(4):
                for wt in warm:
                    nc.tensor.matmul(
                        wt[:, mc, 0:BCH],
                        lhsT=whT_sb[:, 0, mc * 128:(mc + 1) * 128],
                        rhs=h0[:, 0, 0:BCH],
                        start=(mc == 0), stop=(mc == 3),
                        skip_group_check=True)

            h_prev = h0                      # [128, 4(kc), BC] bf16
            h_prev_dt = None
            psa_cur = psS.tile([128, 4, 128], F32, tag="pssA", bufs=2)
            nc.vector.tensor_copy(psa_cur[:, :, 0:BCH], c_sb[:, :, 0:BCH, 0])
            psb_cur = psS.tile([128, 4, 128], F32, tag="pssB", bufs=2)
            nc.vector.tensor_copy(psb_cur[:, :, 0:BCH],
                                  c_sb[:, :, BCH:BC, 0])
            for blk in range(nblk):
                t0 = blk * tblk
                hba = hpool.tile([128, tblk, 4, BCH], BF16, tag="hbA")
                hbb = hpool.tile([128, tblk, 4, BCH], BF16, tag="hbB")
                for dt in range(tblk):
                    t = t0 + dt
                    pa, pb = psa_cur, psb_cur
                    # prefetch step t+1's c for stream A (DVE; queued ahead
                    # of this step's B-act so it never blocks)
                    if t + 1 < T:
                        psa_cur = psS.tile([128, 4, 128], F32, tag="pssA",
                                           bufs=2)
                        nc.vector.tensor_copy(psa_cur[:, :, 0:BCH],
                                              c_sb[:, :, 0:BCH, t + 1])
                    for mc, kc in SLOT2MK:
                        rhs = (h_prev[:, kc, 0:BCH] if h_prev_dt is None
                               else hpa[:, h_prev_dt, kc, :])
                        nc.tensor.matmul(
                            pa[:, mc, 0:BCH],
                            lhsT=whT_sb[:, kc, mc * 128:(mc + 1) * 128],
                            rhs=rhs,
                            start=False, stop=False,
                            skip_group_check=True)
                    # stream A act: single scalar PRELU; round-trip hides
                    # under stream B's 16 matmuls below.
                    nc.scalar.activation(
                        hba[:, dt, :, :], pa[:, :, 0:BCH],
                        mybir.ActivationFunctionType.Prelu, alpha=0.2)
                    # prefetch step t+1's c for stream B (DVE, queued after
                    # stream A's prefetch; done well before B(t+1))
                    if t + 1 < T:
                        psb_cur = psS.tile([128, 4, 128], F32, tag="pssB",
                                           bufs=2)
                        nc.vector.tensor_copy(psb_cur[:, :, 0:BCH],
                                              c_sb[:, :, BCH:BC, t + 1])
                    for mc, kc in SLOT2MK:
                        rhs = (h_prev[:, kc, BCH:BC] if h_prev_dt is None
                               else hpb[:, h_prev_dt, kc, :])
                        nc.tensor.matmul(
                            pb[:, mc, 0:BCH],
                            lhsT=whT_sb[:, kc, mc * 128:(mc + 1) * 128],
                            rhs=rhs,
                            start=False, stop=False,
                            skip_group_check=True)
                    # stream B act: scalar PRELU (queues naturally behind
                    # stream A's, which finishes before B's drain is done);
                    # round-trip hides under stream A's step-t+1 matmuls.
                    nc.scalar.activation(
                        hbb[:, dt, :, :], pb[:, :, 0:BCH],
                        mybir.ActivationFunctionType.Prelu, alpha=0.2)
                    # conv filler: 1 instr at step END, where the PE idles
                    # waiting on PRELU-A(t)'s round-trip before A(t+1)
                    pace["ap"] = hba[:, dt, 0, 0:1]
                    dfill_step()
                    hpa, hpb = hba, hbb
                    h_prev_dt = dt
                # write block: SBUF-native layout, contiguous per partition
                nc.sync.dma_start(
                    out=_dap(out_d, blk * 128 * OWB,
                             [[OWB, 128], [4 * BCH, tblk], [BCH, 4],
                              [1, BCH]]),
                    in_=hba)
                nc.sync.dma_start(
                    out=_dap(out_d, blk * 128 * OWB + tblk * 4 * BCH,
                             [[OWB, 128], [4 * BCH, tblk], [BCH, 4],
                              [1, BCH]]),
                    in_=hbb)
    nc.compile()
    return nc


_nc_cache = {}


def _get_nc(T, tblk):
    key = (T, tblk)
    if key not in _nc_cache:
        _nc_cache[key] = build_nc(T, tblk)
    return _nc_cache[key]


def host_prep(x, A, Bv, W_x, e_x, W_h, W_m, T):
    """Host-side constant prep (float64, exact fn of constant inputs)."""
    order = A.shape[0]
    A64 = A.astype(np.float64)
    b64 = Bv[:, 0].astype(np.float64)
    Hk = np.empty((T, order))
    v = b64.copy()
    for k in range(T):
        Hk[k] = v
        v = A64 @ v
    G = (Hk @ W_m.T.astype(np.float64)).astype(np.float32)      # (T, 512)
    # reverse lag index within each 128-chunk (matches reversed u-shift rows)
    Gr = G.reshape(T // 128, 128, -1)[:, ::-1, :].reshape(T, -1)
    Gr = np.ascontiguousarray(Gr).astype(BF)
    whT = np.ascontiguousarray(W_h.T).astype(BF)
    return Gr, whT


def kernel(x, A, Bv, W_x, e_x, W_h, W_m, T=TFULL, tblk=TBLK):
    x = np.asarray(x, np.float32)
    A = np.asarray(A, np.float32)
    Bv = np.asarray(Bv, np.float32)
    W_x = np.asarray(W_x, np.float32)
    e_x = np.asarray(e_x, np.float32)
    W_h = np.asarray(W_h, np.float32)
    W_m = np.asarray(W_m, np.float32)

    Gr, whT = host_prep(x, A, Bv, W_x, e_x, W_h, W_m, T)
    ident = np.eye(128, dtype=np.float32)

    nc = _get_nc(T, tblk)
    B = x.shape[0]
    nblk = T // tblk
    in_maps = []
    for c in range(NCORES):
        xs = np.ascontiguousarray(
            x[c * BC:(c + 1) * BC, 1:T + 1, :].reshape(BC * T, FEAT))
        in_maps.append({
            "x": xs, "whT": whT, "g": Gr, "wx": W_x.astype(BF),
            "ex": e_x.astype(BF), "ident": ident,
        })
    trace = bool(int(os.environ.get("KERNEL_TRACE", "0")))
    res = run_bass_kernel_spmd(nc, in_maps, list(range(NCORES)), trace=trace)
    last_run_info.clear()
    last_run_info.update(
        exec_time_ns=res.exec_time_ns,
        mean_exec_time_ns=res.mean_exec_time_ns,
        profile_json=res.profile_json,
    )
    out = np.empty((B, T, HID), np.float32)
    for c in range(NCORES):
        o = np.asarray(res.results[c]["out"]).astype(np.float32)
        # [blk*128+p, ((s*tblk+dt)*4+mc)*BCH+b4]
        #   -> [s*BCH+b4, blk*tblk+dt, mc*128+p]
        o = o.reshape(nblk, 128, 2, tblk, 4, BC // 2)
        o = o.transpose(2, 5, 0, 3, 4, 1).reshape(BC, T, HID)
        out[c * BC:(c + 1) * BC] = o
    return out


# revision 37
# speedup vs baseline: 1.0343x; 1.0343x over previous
# Trainium2 Bass kernel for the KerasLMU problem.
#
# Math: per time step t (T=1024 steps),
#   u_t = x_t @ e_x                       (B,1)
#   m_t = m_{t-1} @ A.T + b_row * u_t     (B,256)   -- linear recurrence
#   h_t = lrelu(x_t @ W_x + h_{t-1} @ W_h.T + m_t @ W_m.T)
#
# Reformulation: m_t = sum_k A^k b u_{t-k} (causal convolution), so
#   c_t := x_t @ W_x + m_t @ W_m.T = x_t @ W_x + sum_k G[k] u_{t-k}
# with G[k] = W_m @ (A^k b) precomputed host-side in float64. The only
# sequential device work left is h_t = lrelu(c_t + h_{t-1} @ W_h.T).
#
# Device pipeline (per core, data-parallel over batch; 8 rows/core):
#   A: PE-transpose x -> xT (bf16)          [feat, (b,tau)]
#   B: u = e_x^T @ xT -> u_pad DRAM (bf16, 512 zeros prefix)
#   C: overlapping-window DMA -> ushr (128 shifted copies of u)
#   D: c^T = conv(G,u) + W_x^T xT, bf16 matmuls, f32 PSUM, c kept
#      RESIDENT IN SBUF as bf16 [128, 4, BC, T] (no DRAM round-trip)
#   E: 1024-step recurrence. Per step 16 [128x128]x[128x8] bf16
#      matmuls in a staggered 4-bank slot schedule so each PSUM bank
#      completes ~8 slots before its h-chunk is consumed next step,
#      hiding the PE->act->PE latency (~570ns). Leaky-ReLU alternates
#      between Scalar (Prelu) and Vector (scalar_tensor_tensor) so
#      neither engine serializes the chain. h blocks DMA to DRAM in
#      SBUF-native layout (contiguous 4KB descriptors); the final
#      [b, t, hid] permutation happens host-side in numpy.

import os
import sys

sys.path.insert(0, "/opt/trn_rl_repo")

import numpy as np
import ml_dtypes

import concourse.bass as bass
import concourse.tile as tile
from concourse import bacc, mybir
from concourse.bass_utils import run_bass_kernel_spmd

F32 = mybir.dt.float32
BF16 = mybir.dt.bfloat16
BF = ml_dtypes.bfloat16

NCORES = 8
BATCH = 64
BC = BATCH // NCORES          # batch rows per core = 8
FEAT = 128
HID = 512
ORDER = 256
TFULL = 1024
TBLK = 64                     # seq-loop steps per out-DMA block

# phase-E: the per-core batch of 8 is split into two independent 4-col
# streams, interleaved on the PE. While stream A's activation round-trip
# (~400ns of sem+act latency) is in flight, the PE runs stream B's 16
# matmuls, and vice versa -- the PE never idles, so the recurrence runs at
# PE issue rate (~32 matmuls/step) instead of act-latency rate.
SLOT2MK = [(mc, kc) for kc in range(4) for mc in range(4)]
BCH = BC // 2                 # cols per stream = 4

# module-level stash for test harness introspection
last_run_info = {}


def _dap(handle, offset, dims):
    """Build an explicit AP on a DRAM tensor: dims = [[step, count], ...]
    (element units; first dim pairs with the SBUF partition dim)."""
    base = handle[:]
    return bass.AP(tensor=base.tensor, offset=offset, ap=[list(d) for d in dims])


def build_nc(T=TFULL, tblk=TBLK):
    """Emit the per-core Bass/Tile program (SPMD; all cores identical)."""
    nblk = T // tblk
    BT = BC * T                       # rows of x per core
    nxt = BT // 128                   # 128-row x tiles
    th_n = T // 512 if T >= 512 else 1  # 512-wide tau halves in conv
    tw = min(T, 512)                  # conv tau tile width
    OWB = tblk * 4 * BC               # out elements per partition per block

    nc = bacc.Bacc(None, target_bir_lowering=False)
    x_d = nc.declare_dram_parameter("x", [BT, FEAT], F32, isOutput=False)
    whT_d = nc.declare_dram_parameter("whT", [HID, HID], BF16, isOutput=False)
    g_d = nc.declare_dram_parameter("g", [T, HID], BF16, isOutput=False)
    wx_d = nc.declare_dram_parameter("wx", [FEAT, HID], BF16, isOutput=False)
    ex_d = nc.declare_dram_parameter("ex", [FEAT, 1], BF16, isOutput=False)
    id_d = nc.declare_dram_parameter("ident", [128, 128], F32, isOutput=False)
    # out in SBUF-native block layout: row = blk*128 + p, col = dt*32+mc*8+b8
    out_d = nc.declare_dram_parameter("out", [nblk * 128, OWB], BF16,
                                      isOutput=True)

    UPADW = 512 + T                   # zeros(512) ++ u(T)
    upad_d = nc.dram_tensor("u_pad", [BC, UPADW], BF16)

    USHW = T + 384                    # Qi domain width
    KCN = T // 128                    # lag chunks

    with tile.TileContext(nc) as tc:
        with (
            tc.tile_pool(name="consts", bufs=1) as consts,
            tc.tile_pool(name="work", bufs=4) as work,
            tc.tile_pool(name="hout", bufs=2) as hpool,
            tc.tile_pool(name="psA", bufs=4, space="PSUM") as psA,
            tc.tile_pool(name="psS", bufs=4, space="PSUM") as psS,
        ):
            # ---- resident constants -------------------------------------
            # identity first: the x transposes (critical path) need it
            id_sb = consts.tile([128, 128], F32)
            nc.sync.dma_start(out=id_sb, in_=id_d[:, :])
            ex_sb = consts.tile([128, 1], BF16)
            nc.sync.dma_start(out=ex_sb, in_=ex_d[:, :])

            xT_sb = consts.tile([128, BT], BF16)    # x.T : [feat, (b,tau)]
            # one ushr tile per batch row: the conv for row b8 then depends
            # only on b8's window-DMA, not on all eight
            ushr = [consts.tile([128, USHW], BF16, name=f"ushr{b8}")
                    for b8 in range(BC)]
            c_sb = consts.tile([128, 4, BC, T], BF16)  # c^T resident
            zrow = consts.tile([1, 512], BF16)
            nc.vector.memset(zrow, 0.0)
            h0 = consts.tile([128, 4, BC], BF16)
            nc.vector.memset(h0, 0.0)

            # ---- phases A+B+C, pipelined per batch row ------------------
            # Per b8: stage b8's slice of x (one DMA), PE-transpose it,
            # u = x @ e_x, round-trip through DRAM into the reversed shift
            # matrix ushr[b8][p, Qi] = u_pad[b8][1 + Qi + p]. Row b8's DMAs
            # overlap row b8+1's transposes.
            rpb = T // 128                  # x tiles per batch row
            x_st = [consts.tile([128, rpb, 128], F32, name=f"xst{b8}")
                    for b8 in range(BC)]
            for b8 in range(BC):
                nc.sync.dma_start(
                    out=x_st[b8],
                    in_=_dap(x_d, b8 * T * FEAT,
                             [[FEAT, 128], [128 * FEAT, rpb], [1, FEAT]]))
            # weight loads issue after the x staging (x gates the critical
            # path; whT/g aren't needed until the conv/recurrence)
            whT_sb = consts.tile([128, 4, HID], BF16)
            for kc in range(4):
                nc.sync.dma_start(out=whT_sb[:, kc, :],
                                  in_=whT_d[kc * 128:(kc + 1) * 128, :])
            g_sb = consts.tile([128, KCN, HID], BF16)
            for kc in range(KCN):
                nc.sync.dma_start(out=g_sb[:, kc, :],
                                  in_=g_d[kc * 128:(kc + 1) * 128, :])
            wx_sb = consts.tile([128, HID], BF16)
            nc.sync.dma_start(out=wx_sb, in_=wx_d[:, :])
            for b8 in range(BC):
                for r in range(rpb):
                    ps = psA.tile([128, 128], F32, tag="ps")
                    nc.tensor.transpose(ps, x_st[b8][:, r, :], id_sb)
                    dst = xT_sb[:, (b8 * rpb + r) * 128:
                                (b8 * rpb + r + 1) * 128]
                    if r % 2 == 0:
                        nc.scalar.copy(dst, ps)
                    else:
                        nc.vector.tensor_copy(dst, ps)
                urow = work.tile([1, UPADW], BF16, tag="urow")
                nc.vector.tensor_copy(urow[:, 0:512], zrow)
                for th in range((T + 511) // 512):
                    w = min(512, T - th * 512)
                    ps = psA.tile([1, 512], F32, tag="ps")
                    nc.tensor.matmul(ps[:, :w], lhsT=ex_sb,
                                     rhs=xT_sb[:, b8 * T + th * 512:
                                               b8 * T + th * 512 + w],
                                     start=True, stop=True)
                    nc.scalar.copy(urow[:, 512 + th * 512:512 + th * 512 + w],
                                   ps[:, :w])
                nc.gpsimd.dma_start(out=upad_d[b8:b8 + 1, :], in_=urow)
                nc.sync.dma_start(
                    out=ushr[b8],
                    in_=_dap(upad_d, b8 * UPADW + 1, [[1, 128], [1, USHW]]))

            # ---- phase D: c^T = conv(G, u) + W_x^T @ x^T -> c_sb (SBUF) -
            # 512-column tau tiles. Lag chunk kc contributes only to taus
            # >= 128*kc (earlier taus fall in the zero prefix of u_pad), so
            # each matmul's tau range is trimmed. th=0 (taus 0..511, needed
            # before E step 0) runs in the prologue; th=1 groups drip into
            # phase E's PE idle gap at 1 instruction per step.
            def dgroup_gen(b8, jt, th, scalar_copy=False):
                ps = psA.tile([128, tw], F32, tag="ps",
                              name=f"cps{b8}_{jt}_{th}")
                first = True
                kmax = min(KCN, 4 * th + tw // 128)
                for kc in range(kmax):
                    s0 = max(0, 128 * kc - 512 * th)
                    qi0 = 384 + 512 * th - 128 * kc
                    nc.tensor.matmul(
                        ps[:, s0:tw],
                        lhsT=g_sb[:, kc, jt * 128:(jt + 1) * 128],
                        rhs=ushr[b8][:, qi0 + s0:qi0 + tw],
                        start=first, stop=False)
                    first = False
                    yield
                nc.tensor.matmul(
                    ps, lhsT=wx_sb[:, jt * 128:(jt + 1) * 128],
                    rhs=xT_sb[:, b8 * T + th * 512:b8 * T + th * 512 + tw],
                    start=False, stop=True)
                yield
                if scalar_copy:
                    nc.scalar.copy(c_sb[:, jt, b8, th * 512:th * 512 + tw],
                                   ps)
                    yield
                else:
                    # split halves across scalar+vector so neither engine
                    # saturates when this lands inside a phase-E step
                    hw2 = tw // 2
                    o0 = th * 512
                    nc.scalar.copy(c_sb[:, jt, b8, o0:o0 + hw2],
                                   ps[:, 0:hw2])
                    yield
                    nc.vector.tensor_copy(c_sb[:, jt, b8, o0 + hw2:o0 + tw],
                                          ps[:, hw2:tw])
                    yield

            ev = 0
            for b8 in range(BC):
                for jt in range(4):
                    for _ in dgroup_gen(b8, jt, 0, scalar_copy=ev % 2 == 0):
                        pass
                    ev += 1
            dfill = []
            if th_n == 2:
                for b8 in range(BC):
                    for jt in range(4):
                        dfill.append(dgroup_gen(b8, jt, 1))

            def dfill_step():
                while dfill:
                    try:
                        next(dfill[0])
                        return
                    except StopIteration:
                        dfill.pop(0)

            # ---- phase E: sequential h recurrence -----------------------
            # Warm all psS banks once: a start=True pass clears the
            # pending-zero bits over our regions so the per-step matmuls
            # can run start=False and accumulate onto a prewritten c_t
            # (keeps the c add off the PE critical path).
            # Every per-stream object (PSUM step tile, h output tile) is a
            # SEPARATE tile: dependency tracking at any granularity then
            # cannot couple the two streams, so their act round-trips
            # overlap with the other stream's matmuls. PSUM step tiles are
            # padded to a full 2KB bank (a bank holds one active
            # accumulation group); bufs=2 per stream -> 4 banks.
            warm = [psS.tile([128, 4, 128], F32, tag=f"pss{s}", bufs=2,
                             name=f"warm{s}{i}")
                    for s in "AB" for i in range(2)]
            for mc in range(4):
                for wt in warm:
                    nc.tensor.matmul(
                        wt[:, mc, 0:BCH],
                        lhsT=whT_sb[:, 0, mc * 128:(mc + 1) * 128],
                        rhs=h0[:, 0, 0:BCH],
                        start=(mc == 0), stop=(mc == 3),
                        skip_group_check=True)

            h_prev = h0                      # [128, 4(kc), BC] bf16
            h_prev_dt = None
            psa_cur = psS.tile([128, 4, 128], F32, tag="pssA", bufs=2)
            nc.vector.tensor_copy(psa_cur[:, :, 0:BCH], c_sb[:, :, 0:BCH, 0])
            psb_cur = psS.tile([128, 4, 128], F32, tag="pssB", bufs=2)
            nc.vector.tensor_copy(psb_cur[:, :, 0:BCH],
                                  c_sb[:, :, BCH:BC, 0])
            for blk in range(nblk):
                t0 = blk * tblk
                hba = hpool.tile([128, tblk, 4, BCH], BF16, tag="hbA")
                hbb = hpool.tile([128, tblk, 4, BCH], BF16, tag="hbB")
                for dt in range(tblk):
                    t = t0 + dt
                    pa, pb = psa_cur, psb_cur
                    # prefetch step t+1's c for stream A (DVE; queued ahead
                    # of this step's B-act so it never blocks)
                    if t + 1 < T:
                        psa_cur = psS.tile([128, 4, 128], F32, tag="pssA",
                                           bufs=2)
                        nc.vector.tensor_copy(psa_cur[:, :, 0:BCH],
                                              c_sb[:, :, 0:BCH, t + 1])
                    for mc, kc in SLOT2MK:
                        rhs = (h_prev[:, kc, 0:BCH] if h_prev_dt is None
                               else hpa[:, h_prev_dt, kc, :])
                        nc.tensor.matmul(
                            pa[:, mc, 0:BCH],
                            lhsT=whT_sb[:, kc, mc * 128:(mc + 1) * 128],
                            rhs=rhs,
                            start=False, stop=False,
                            skip_group_check=True)
                    # stream A act: single scalar PRELU; round-trip hides
                    # under stream B's 16 matmuls below.
                    nc.scalar.activation(
                        hba[:, dt, :, :], pa[:, :, 0:BCH],
                        mybir.ActivationFunctionType.Prelu, alpha=0.2)
                    # prefetch step t+1's c for stream B (DVE, queued after
                    # stream A's prefetch; done well before B(t+1))
                    if t + 1 < T:
                        psb_cur = psS.tile([128, 4, 128], F32, tag="pssB",
                                           bufs=2)
                        nc.vector.tensor_copy(psb_cur[:, :, 0:BCH],
                                              c_sb[:, :, BCH:BC, t + 1])
                    for mc, kc in SLOT2MK:
                        rhs = (h_prev[:, kc, BCH:BC] if h_prev_dt is None
                               else hpb[:, h_prev_dt, kc, :])
                        nc.tensor.matmul(
                            pb[:, mc, 0:BCH],
                            lhsT=whT_sb[:, kc, mc * 128:(mc + 1) * 128],
                            rhs=rhs,
                            start=False, stop=False,
                            skip_group_check=True)
                    # stream B act: scalar PRELU (queues naturally behind
                    # stream A's, which finishes before B's drain is done);
                    # round-trip hides under stream A's step-t+1 matmuls.
                    nc.scalar.activation(
                        hbb[:, dt, :, :], pb[:, :, 0:BCH],
                        mybir.ActivationFunctionType.Prelu, alpha=0.2)
                    # conv filler: 1 instr at step END, where the PE idles
                    # waiting on PRELU-A(t)'s round-trip before A(t+1)
                    dfill_step()
                    hpa, hpb = hba, hbb
                    h_prev_dt = dt
                # write block: SBUF-native layout, contiguous per partition
                nc.sync.dma_start(
                    out=_dap(out_d, blk * 128 * OWB,
                             [[OWB, 128], [4 * BCH, tblk], [BCH, 4],
                              [1, BCH]]),
                    in_=hba)
                nc.sync.dma_start(
                    out=_dap(out_d, blk * 128 * OWB + tblk * 4 * BCH,
                             [[OWB, 128], [4 * BCH, tblk], [BCH, 4],
                              [1, BCH]]),
                    in_=hbb)
    nc.compile()
    return nc


_nc_cache = {}


def _get_nc(T, tblk):
    key = (T, tblk)
    if key not in _nc_cache:
        _nc_cache[key] = build_nc(T, tblk)
    return _nc_cache[key]


def host_prep(x, A, Bv, W_x, e_x, W_h, W_m, T):
    """Host-side constant prep (float64, exact fn of constant inputs)."""
    order = A.shape[0]
    A64 = A.astype(np.float64)
    b64 = Bv[:, 0].astype(np.float64)
    Hk = np.empty((T, order))
    v = b64.copy()
    for k in range(T):
        Hk[k] = v
        v = A64 @ v
    G = (Hk @ W_m.T.astype(np.float64)).astype(np.float32)      # (T, 512)
    # reverse lag index within each 128-chunk (matches reversed u-shift rows)
    Gr = G.reshape(T // 128, 128, -1)[:, ::-1, :].reshape(T, -1)
    Gr = np.ascontiguousarray(Gr).astype(BF)
    whT = np.ascontiguousarray(W_h.T).astype(BF)
    return Gr, whT


def kernel(x, A, Bv, W_x, e_x, W_h, W_m, T=TFULL, tblk=TBLK):
    x = np.asarray(x, np.float32)
    A = np.asarray(A, np.float32)
    Bv = np.asarray(Bv, np.float32)
    W_x = np.asarray(W_x, np.float32)
    e_x = np.asarray(e_x, np.float32)
    W_h = np.asarray(W_h, np.float32)
    W_m = np.asarray(W_m, np.float32)

    Gr, whT = host_prep(x, A, Bv, W_x, e_x, W_h, W_m, T)
    ident = np.eye(128, dtype=np.float32)

    nc = _get_nc(T, tblk)
    B = x.shape[0]
    nblk = T // tblk
    in_maps = []
    for c in range(NCORES):
        xs = np.ascontiguousarray(
            x[c * BC:(c + 1) * BC, 1:T + 1, :].reshape(BC * T, FEAT))
        in_maps.append({
            "x": xs, "whT": whT, "g": Gr, "wx": W_x.astype(BF),
            "ex": e_x.astype(BF), "ident": ident,
        })
    trace = bool(int(os.environ.get("KERNEL_TRACE", "0")))
    res = run_bass_kernel_spmd(nc, in_maps, list(range(NCORES)), trace=trace)
    last_run_info.clear()
    last_run_info.update(
        exec_time_ns=res.exec_time_ns,
        mean_exec_time_ns=res.mean_exec_time_ns,
        profile_json=res.profile_json,
    )
    out = np.empty((B, T, HID), np.float32)
    for c in range(NCORES):
        o = np.asarray(res.results[c]["out"]).astype(np.float32)
        # [blk*128+p, ((s*tblk+dt)*4+mc)*BCH+b4]
        #   -> [s*BCH+b4, blk*tblk+dt, mc*128+p]
        o = o.reshape(nblk, 128, 2, tblk, 4, BC // 2)
        o = o.transpose(2, 5, 0, 3, 4, 1).reshape(BC, T, HID)
        out[c * BC:(c + 1) * BC] = o
    return out
